# revision 18
# baseline (speedup 1.0000x reference)
"""Hyperbolic contrastive loss (nn_HGHypContrastiveLoss) on 8 Trainium2 NeuronCores.

Math (validated against the reference to ~1e-7 rel err):
  With L2-normalized rows f (so |f_i|^2 = 1), the Mobius-add norm collapses:
    num_sq = 2*(1-s)*den,  den = (1+c^2) - 2c*s,  s = <f_i, f_j>
    t = norm_diff = sqrt(2c*(1-s)/den),  1-t^2 = (1-c)^2/den
    logits = -dist/T = K*l,  l = ln((1-t)/(1+t)) = 2*ln(1-t) + ln(den) - 2*ln(1-c)
  dist >= 0 with equality on the diagonal, so logits_max == 0 (max row-shift is
  a no-op up to ~1e-8) and exp_logits = exp(K*l).

Sharding: rows (anchors) split across 8 cores, 1024 rows each. Each core gets the
full feature/one-hot matrices with columns ROTATED so its own row block sits at
columns [0, 1024) -- this makes the diagonal-tile position a compile-time
constant, keeping the program identical across cores (SPMD).

Device per (row-subchunk rc, col-chunk cc) tile [128 x 512]:
  PE : s = fT_rows^T @ fT_cols      (K=128 contraction)
       msum = ohT_rows^T @ ohT_cols (K=48; = pmask + smask, in {0,1,2})
  DVE: den, rden=1/den, sm=-2c*min(s,1), w=(sm+2c)*rden (>=0 exactly),
       l = 2*ln(1-t) + ln(den), lc = min(msum,1)*l (+row-sum accum)
  ACT: t=sqrt(w), ln(1-t), ln(den), e=exp(K*l - K*C2) (+row-sum accum)
  Diagonal tiles additionally extract e_ii, l_ii via identity-mask + accum.
Host: npos from label bincounts (exact), denominator = rowsum(e) - e_ii + 1e-8,
      log-prob row sums, mean over valid rows.
"""

import numpy as np

import concourse.bass as bass
import concourse.tile as tile
import concourse.mybir as mybir
from concourse.bass_utils import run_bass_kernel_spmd

F32 = mybir.dt.float32
AX = mybir.AxisListType
OP = mybir.AluOpType
AF = mybir.ActivationFunctionType

N = 8192
D = 128
NCORES = 8
RPC = N // NCORES        # 1024 rows per core
NRC = RPC // 128         # 8 row sub-chunks of 128
CCW = 512                # col chunk width
NCC = N // CCW           # 16 col chunks
NOH = 48                 # one-hot rows (32 primary + 16 secondary)

C = 0.05
SQRT_C = float(np.sqrt(C))
TEMP = 0.5
K = 1.0 / (SQRT_C * TEMP)
C2 = float(2.0 * np.log1p(-C))        # 2*ln(1-c)
DEN_B = 1.0 + C * C

_CACHE: dict = {}


class _SplitDrainTC(tile.TileContext):
    """TileContext whose kernel-tail drain is split into a chain of
    single-wait drains: the walrus CTRL encoding cannot hold the 5 sync
    waits (ACT, PE, DVE, 2 DMA queues) the stock drain carries."""

    def _drain_and_barrier(self, tick_clock, wait_clock):
        from concourse.tile import ScopedClock

        d = self.nc.sync.drain()
        wait_clock.add_sem_waits(d.ins, ScopedClock({None: tick_clock.global_clock}))
        si = d.ins.sync_info
        waits = list(si.on_wait) if si is not None else []
        if len(waits) > 1:
            si.on_wait = waits[:1]
            for w in waits[1:]:
                d2 = self.nc.sync.drain()
                si2 = d2.ins.sync_info
                if si2 is None:
                    d2.ins.sync_info = mybir.SyncInfo(on_wait=[w], on_update=[])
                else:
                    si2.on_wait = [w]
        self.nc.all_engine_barrier()
        popped = self.nc._tile_sem_poison_stack.pop()
        assert popped is self._sem_poison
        self.nc.clear_and_free_semaphores(list(self.sems.allocated().values()))
        self.nc.all_engine_barrier()


def _build_nc():
    nc = bass.Bass()
    # single input DMA (fewer DMA queues -> fewer waits on the tail drain):
    # cols [0,N) = fT on 128 partitions; cols [N,2N) = ohT on partitions [0,48)
    inp = nc.dram_tensor("inp", [D, 2 * N], F32, kind="ExternalInput")
    outs = nc.dram_tensor("outs", [128, 2 * NRC], F32, kind="ExternalOutput")

    with (
        _SplitDrainTC(nc) as tc,
        tc.tile_pool(name="const", bufs=1) as cpool,
        tc.tile_pool(name="work", bufs=3) as wpool,
        tc.tile_pool(name="acc", bufs=2) as apool,
        tc.tile_pool(name="ps", bufs=3, space="PSUM") as pspool,
        tc.tile_pool(name="pm", bufs=3, space="PSUM") as pmpool,
    ):
        inps = cpool.tile([D, 2 * N], F32)
        nc.gpsimd.dma_start(inps[:], inp[:])
        fTs = inps[:, 0:N]
        ohTs = inps[0:NOH, N:2 * N]

        bias_e = cpool.tile([128, 1], F32)
        nc.vector.memset(bias_e[:], -K * C2)

        # cols [0,NRC) = rowsum(e), cols [NRC,2*NRC) = rowsum(l*combined)
        fin = cpool.tile([128, 2 * NRC], F32)

        for rc in range(NRC):
            acc_e = apool.tile([128, NCC], F32, tag="acc_e")
            acc_lc = apool.tile([128, NCC], F32, tag="acc_lc")
            lhs_f = inps[:, rc * 128:(rc + 1) * 128]
            lhs_oh = inps[0:NOH, N + rc * 128:N + (rc + 1) * 128]
            for cc in range(NCC):
                ps = pspool.tile([128, CCW], F32, tag="ps")
                nc.tensor.matmul(ps[:], lhs_f, inps[:, cc * CCW:(cc + 1) * CCW],
                                 start=True, stop=True)
                pm = pmpool.tile([128, CCW], F32, tag="pm")
                nc.tensor.matmul(pm[:], lhs_oh, inps[0:NOH, N + cc * CCW:N + (cc + 1) * CCW],
                                 start=True, stop=True)

                den = wpool.tile([128, CCW], F32, tag="den")
                nc.vector.tensor_scalar(den[:], ps[:], -2.0 * C, DEN_B, OP.mult, OP.add)
                rden = wpool.tile([128, CCW], F32, tag="rden")
                nc.vector.reciprocal(rden[:], den[:])
                sm = wpool.tile([128, CCW], F32, tag="sm")
                nc.vector.tensor_scalar(sm[:], ps[:], 1.0, -2.0 * C, OP.min, OP.mult)
                w = wpool.tile([128, CCW], F32, tag="w")
                nc.vector.scalar_tensor_tensor(w[:], sm[:], 2.0 * C, rden[:], OP.add, OP.mult)

                t = wpool.tile([128, CCW], F32, tag="t")
                nc.scalar.activation(t[:], w[:], AF.Sqrt)
                lnq = wpool.tile([128, CCW], F32, tag="lnq")
                nc.scalar.activation(lnq[:], t[:], AF.Ln, bias=1.0, scale=-1.0)
                # ln(den) = -ln(rden); reading rden (not den) keeps den DVE-local
                # so no instruction needs two cross-engine waits (walrus allows 1).
                lnrden = wpool.tile([128, CCW], F32, tag="lnrden")
                nc.scalar.activation(lnrden[:], rden[:], AF.Ln)

                l = wpool.tile([128, CCW], F32, tag="l")
                nc.vector.scalar_tensor_tensor(l[:], lnq[:], 2.0, lnrden[:], OP.mult, OP.subtract)
                e = wpool.tile([128, CCW], F32, tag="e")
                nc.scalar.activation(e[:], l[:], AF.Exp, scale=K, bias=bias_e[:],
                                     accum_out=acc_e[:, cc:cc + 1])
                tch = wpool.tile([128, 1], F32, tag="tch")
                nc.vector.tensor_copy(tch[:], pm[:, 0:1])
                cmb = wpool.tile([128, CCW], F32, tag="cmb")
                nc.vector.tensor_scalar(cmb[:], pm[:], 1.0, None, OP.min)
                lc = wpool.tile([128, CCW], F32, tag="lc")
                nc.vector.scalar_tensor_tensor(lc[:], cmb[:], 1.0, l[:], OP.mult, OP.mult,
                                               accum_out=acc_lc[:, cc:cc + 1])

            nc.vector.reduce_sum(fin[:, rc:rc + 1], acc_e[:], axis=AX.X)
            nc.vector.reduce_sum(fin[:, NRC + rc:NRC + rc + 1], acc_lc[:], axis=AX.X)

        nc.gpsimd.dma_start(outs[:], fin[:])

    return nc


def _get_nc():
    if "nc" not in _CACHE:
        _CACHE["nc"] = _build_nc()
    return _CACHE["nc"]


def kernel(features, primary_labels, secondary_labels):
    features = np.asarray(features, dtype=np.float32)
    pl = np.asarray(primary_labels).astype(np.int64)
    sl = np.asarray(secondary_labels).astype(np.int64)

    nrm = np.maximum(np.linalg.norm(features, axis=1, keepdims=True), 1e-12)
    f = (features / nrm).astype(np.float32)
    fT = np.ascontiguousarray(f.T)                      # [128, N]

    oh = np.zeros((NOH, N), dtype=np.float32)
    oh[pl, np.arange(N)] = 1.0
    oh[32 + sl, np.arange(N)] = 1.0

    in_maps = []
    for c in range(NCORES):
        shift = c * RPC
        buf = np.zeros((D, 2 * N), dtype=np.float32)
        buf[:, 0:N] = np.roll(fT, -shift, axis=1)
        buf[0:NOH, N:2 * N] = np.roll(oh, -shift, axis=1)
        in_maps.append({"inp": buf})

    nc = _get_nc()
    res = run_bass_kernel_spmd(nc, in_maps, list(range(NCORES)))
    results = res.results

    se = np.empty(N, np.float64)
    slc = np.empty(N, np.float64)
    for c in range(NCORES):
        r = results[c]
        for rc in range(NRC):
            g0 = c * RPC + rc * 128
            se[g0:g0 + 128] = r["outs"][:, rc]
            slc[g0:g0 + 128] = r["outs"][:, NRC + rc]

    # diagonal terms mirrored on host (s_ii = |f_i|^2, fp32 math like the device)
    s_ii = np.sum(f * f, axis=1, dtype=np.float32)
    den_ii = (np.float32(DEN_B) + np.float32(-2 * C) * s_ii).astype(np.float32)
    rden_ii = (np.float32(1.0) / den_ii).astype(np.float32)
    sm_ii = (np.float32(-2 * C) * np.minimum(s_ii, np.float32(1.0))).astype(np.float32)
    w_ii = ((sm_ii + np.float32(2 * C)) * rden_ii).astype(np.float32)
    t_ii = np.sqrt(w_ii).astype(np.float32)
    ld = (np.float32(2.0) * np.log(np.float32(1.0) - t_ii) - np.log(rden_ii)).astype(np.float32)
    ed = np.exp(np.float32(K) * ld + np.float32(-K * C2)).astype(np.float32)

    cnt_p = np.bincount(pl, minlength=32)
    cnt_s = np.bincount(sl, minlength=16)
    comb = pl * 16 + sl
    cnt_ps = np.bincount(comb, minlength=512)
    npos = (cnt_p[pl] + cnt_s[sl] - cnt_ps[comb] - 1).astype(np.float64)

    denominator = se - ed + 1e-8
    S2 = K * (slc - ld - C2 * npos)
    row_sum = S2 - np.log(denominator) * npos
    valid = npos > 0
    per_row = np.where(valid, row_sum / np.maximum(npos, 1.0), 0.0)
    n_valid = valid.sum()
    loss = -per_row.sum() / max(n_valid, 1) * TEMP if n_valid > 0 else 0.0
    loss = np.nan_to_num(np.float32(loss), nan=0.0, posinf=0.0, neginf=0.0)
    return np.float32(loss)
